# revision 10
# baseline (speedup 1.0000x reference)
"""Trainium2 Bass kernel for nn_MixedRepeatHeads.

Computation (full shapes):
  proj[h,b,k] = einsum(x[b,d], proj_w[h,k,d]) + proj_b[h,k]
  w = mix_w[:, index]; bb = mix_b[:, index]
  decay = clip(decay_value, 0.9, 1.0) ** (1/8)
  coef[h] = w*decay (h<8) else decay
  hidden[b, h*256+k] = w[h]*proj[h,b,k] + coef[h]*cache[h,b,k] + bb[h]
  out = hidden @ out_w.T + out_b                     # [8192, 4096]

Algebraic refold (host side, cheap):
  out = x @ C.T + (coef*cache) @ out_w.T + ob2
    C   = out_w @ (w*proj_w)          (one 4096^3 host GEMM)
    ob2 = out_b + out_w @ (w*proj_b + bb)
  => ONE GEMM with K=8192:  out = A @ Wcat + ob2
    A    = [x | coef*cache]           # [8192, 8192]
    Wcat = [C.T ; out_w.T]            # [8192, 4096]

Device: data-parallel over batch across 8 cores (1024 rows/core), all-bf16
matmuls (N=512 moving, fp32 PSUM accumulation; bf16 hides PE weight loads,
unlike f32r whose matmuls must self-load serially).  The batch is processed
as two 512-column halves, each a full pass over the 32 j-tiles streaming the
weights; each half's activation tile reload (WAR-ordered by the tile pool)
overlaps the opposite half's entire compute pass, so steady-state has no
activation stalls and DMA queues never head-of-line block the weight stream.
All DRAM layouts are partition-major so every DMA descriptor is a large
contiguous run per partition.
Device output is outT [4096, 1024] f32 per core; host transposes + concats.
"""

import sys

if "/opt/trn_rl_repo" not in sys.path:
    sys.path.insert(0, "/opt/trn_rl_repo")

import contextlib

import numpy as np
import ml_dtypes

import bass_rust
import concourse.bass as bass
import concourse.tile as tile
from concourse import mybir
from concourse.bass_utils import run_bass_kernel_spmd
from concourse.vector_clock import ScopedClock

# ---------------------------------------------------------------- constants
N_HEADS = 16
HIDDEN = 256
DIM = 4096
BATCH = 8192
DECAY_CONSTANT = 8
N_CORES = 8
BC = BATCH // N_CORES  # 1024 batch rows per core
HALF = BC // 2  # 512
P = 128
KDIM = 2 * DIM  # 8192 contraction (x ++ cache)
KT = KDIM // P  # 64 k-tiles
JT = DIM // P  # 32 j-tiles

F32 = mybir.dt.float32
BF16 = mybir.dt.bfloat16
NP_BF16 = ml_dtypes.bfloat16

# ------------------------------------------------- walrus wait legalization
# This walrus build supports only ONE sync-wait command per instruction.
MAXW = 1


class SafeTileContext(tile.TileContext):
    def _split_waits_in_ordered(self, ordered):
        nc = self.nc
        for _bb_name, insts in ordered.items():
            new_list = []
            changed = False
            for inst in insts:
                si = inst.sync_info
                if si is not None and len(si.on_wait) > MAXW:
                    waits = list(si.on_wait)
                    ups = list(si.on_update)
                    head, tail = waits[:-MAXW], waits[-MAXW:]
                    for w in head:
                        nop = mybir.InstNoOp(
                            name=nc.get_next_instruction_name(),
                            engine=inst.engine,
                            ins=[],
                            outs=[],
                            sync_info=bass_rust.SyncInfo(on_wait=[w], on_update=[]),
                            bass_nofuse=True,
                        )
                        nc.register_instruction(nop, overwrite=True)
                        new_list.append(nop)
                    inst.sync_info = bass_rust.SyncInfo(on_wait=tail, on_update=ups)
                    changed = True
                new_list.append(inst)
            if changed:
                insts[:] = new_list
        return ordered

    def _lower_ordered_insts(self, ordered):
        self._split_waits_in_ordered(ordered)
        return super()._lower_ordered_insts(ordered)

    def _drain_and_barrier(self, tick_clock, wait_clock):
        probe = self.nc.sync.nop(nofuse=True)
        wait_clock.add_sem_waits(
            probe.ins, ScopedClock({None: tick_clock.global_clock})
        )
        si = probe.ins.sync_info
        waits = list(si.on_wait) if si is not None else []
        upd = list(si.on_update) if si is not None else []
        probe.ins.sync_info = bass_rust.SyncInfo(on_wait=waits[:MAXW], on_update=upd)
        for i in range(MAXW, len(waits), MAXW):
            n = self.nc.sync.nop(nofuse=True)
            n.ins.sync_info = bass_rust.SyncInfo(
                on_wait=waits[i : i + MAXW], on_update=[]
            )

        self.nc.sync.drain()

        self.nc.all_engine_barrier()
        assert self.sems is not None
        popped = self.nc._tile_sem_poison_stack.pop()
        assert popped is self._sem_poison
        self.nc.clear_and_free_semaphores(list(self.sems.allocated().values()))
        self.nc.all_engine_barrier()


# ------------------------------------------------------------ kernel build
def build_kernel(loop_reps=None):
    """Per-core program. DRAM params:
      a    [2, P, KT, HALF] bf16 : a[h, p, kt, b] = A[h*512+b, kt*128+p]
      wt   [JT, P, KT, P]   bf16 : wt[jt, p, kt, j] = Wcat[kt*128+p, jt*128+j]
      ob   [P, JT]          f32  : ob[p, jt] = ob2[jt*128+p]
      outT [DIM, BC]        bf16 : output, transposed (j-major)
    """
    nc = bass.Bass()
    a = nc.declare_dram_parameter("a", [2, P, KT, HALF], BF16, isOutput=False)
    wt = nc.declare_dram_parameter("wt", [JT, P, KT, P], BF16, isOutput=False)
    ob = nc.declare_dram_parameter("ob", [P, JT], F32, isOutput=False)
    outT = nc.declare_dram_parameter("outT", [DIM, BC], BF16, isOutput=True)

    ACH = 8  # a-half DMA chunks (8 kt per chunk, contiguous per partition)
    KCH = KT // ACH

    with SafeTileContext(nc) as tc:
        with (
            tc.tile_pool(name="a0pool", bufs=1) as a0pool,
            tc.tile_pool(name="a1pool", bufs=1) as a1pool,
            tc.tile_pool(name="wpool", bufs=4) as wpool,
            tc.tile_pool(name="opool", bufs=3) as opool,
            tc.tile_pool(name="obpool", bufs=1) as obpool,
            tc.tile_pool(name="pspool", bufs=6, space="PSUM") as pspool,
        ):
            # SP HWDGE queue is reserved for the weight stream; activations,
            # bias, and outputs ride the Activation HWDGE queue instead.
            ob_t = obpool.tile([P, JT], F32)
            nc.scalar.dma_start(ob_t[:], ob[:])

            loop_cm = (
                tc.For_i(0, loop_reps, 1)
                if loop_reps is not None
                else contextlib.nullcontext()
            )
            with loop_cm:
                for half in range(2):
                    apool = a0pool if half == 0 else a1pool
                    a_t = apool.tile([P, KT, HALF], BF16, tag=f"a{half}")
                    for c in range(ACH):
                        ksl = slice(c * KCH, (c + 1) * KCH)
                        nc.scalar.dma_start(a_t[:, ksl, :], a[half][:, ksl, :])
                    for jt in range(JT):
                        wblk = wpool.tile([P, KT, P], BF16, tag="wblk")
                        nc.sync.dma_start(wblk[:], wt[jt])
                        ps = pspool.tile([P, HALF], F32, tag="ps")
                        for kt in range(KT):
                            nc.tensor.matmul(
                                ps[:],
                                wblk[:, kt, :],
                                a_t[:, kt, :],
                                start=(kt == 0),
                                stop=(kt == KT - 1),
                            )
                        o_t = opool.tile([P, HALF], BF16, tag="o")
                        nc.vector.tensor_tensor(
                            o_t[:],
                            ps[:],
                            ob_t[:, jt : jt + 1].to_broadcast((P, HALF)),
                            mybir.AluOpType.add,
                        )
                        nc.scalar.dma_start(
                            outT[
                                jt * P : (jt + 1) * P,
                                half * HALF : (half + 1) * HALF,
                            ],
                            o_t[:],
                        )

    return nc


# ------------------------------------------------------------- host helpers
def _host_prepare(inputs):
    x = np.asarray(inputs["x"], dtype=np.float32)
    proj_w = np.asarray(inputs["proj_w"], dtype=np.float32)
    proj_b = np.asarray(inputs["proj_b"], dtype=np.float32)
    mix_w = np.asarray(inputs["mix_w"], dtype=np.float32)
    mix_b = np.asarray(inputs["mix_b"], dtype=np.float32)
    decay_value = np.asarray(inputs["decay_value"], dtype=np.float32)
    cache = np.asarray(inputs["cache"], dtype=np.float32)
    out_w = np.asarray(inputs["out_w"], dtype=np.float32)
    out_b = np.asarray(inputs["out_b"], dtype=np.float32)
    idx = int(np.asarray(inputs["index"]))

    w = mix_w[:, idx]  # [16]
    bb = mix_b[:, idx]  # [16]
    decay = np.clip(decay_value, 0.9, 1.0) ** np.float32(1.0 / DECAY_CONSTANT)
    is_col = np.arange(N_HEADS) < (N_HEADS // 2)
    coef = np.where(is_col, w * decay, decay).astype(np.float32)  # [16]

    PW = (proj_w * w[:, None, None]).reshape(DIM, DIM)  # [i, d]
    C = out_w @ PW  # [j, d]
    bias_i = (w[:, None] * proj_b + bb[:, None]).reshape(DIM)  # [i]
    ob2 = out_b + out_w @ bias_i  # [j]

    # Wcat [K=8192, j]: rows 0..4095 = C.T, rows 4096.. = out_w.T
    Wcat = np.empty((KDIM, DIM), dtype=NP_BF16)
    Wcat[:DIM] = C.T.astype(NP_BF16)
    Wcat[DIM:] = out_w.T.astype(NP_BF16)
    WT = np.ascontiguousarray(Wcat.reshape(KT, P, JT, P).transpose(2, 1, 0, 3))

    # A.T [K=8192, batch]: rows 0..4095 = x.T, rows 4096.. = (coef*cache).T
    AT = np.empty((KDIM, BATCH), dtype=NP_BF16)
    AT[:DIM] = x.T.astype(NP_BF16)
    cc = (cache * coef[:, None, None]).astype(NP_BF16)  # [h, b, k]
    AT[DIM:] = cc.transpose(0, 2, 1).reshape(DIM, BATCH)
    # [kt, p, core, half, b'] -> per-core [2, P, KT, HALF] (partition-major)
    AT_r = AT.reshape(KT, P, N_CORES, 2, HALF).transpose(2, 3, 1, 0, 4)

    obT = np.ascontiguousarray(ob2.reshape(JT, P).T)  # [P, JT]

    in_maps = []
    for c in range(N_CORES):
        in_maps.append(
            {
                "a": np.ascontiguousarray(AT_r[c]),
                "wt": WT,
                "ob": obT,
            }
        )
    return in_maps


def _assemble(results):
    # results: list per core of {"outT": [DIM, BC]}
    out = np.empty((BATCH, DIM), dtype=np.float32)
    for c in range(N_CORES):
        out[c * BC : (c + 1) * BC] = results[c]["outT"].T.astype(np.float32)
    return out


_NC_CACHE = None


def _get_nc():
    global _NC_CACHE
    if _NC_CACHE is None:
        _NC_CACHE = build_kernel()
    return _NC_CACHE


def kernel(**inputs) -> np.ndarray:
    in_maps = _host_prepare(inputs)
    nc = _get_nc()
    res = run_bass_kernel_spmd(nc, in_maps, list(range(N_CORES)))
    return _assemble(res.results)


if __name__ == "__main__":
    # quick self-run with random data of the right shapes
    rng = np.random.default_rng(0)
    ins = {
        "x": rng.standard_normal((BATCH, DIM), dtype=np.float32),
        "proj_w": rng.standard_normal((N_HEADS, HIDDEN, DIM), dtype=np.float32) * 0.02,
        "proj_b": rng.standard_normal((N_HEADS, HIDDEN), dtype=np.float32) * 0.02,
        "mix_w": rng.standard_normal((N_HEADS, 4096), dtype=np.float32) * 0.02 + 1.0,
        "mix_b": rng.standard_normal((N_HEADS, 4096), dtype=np.float32) * 0.02,
        "decay_value": rng.uniform(0.85, 1.05, size=(N_HEADS,)).astype(np.float32),
        "cache": rng.standard_normal((N_HEADS, BATCH, HIDDEN), dtype=np.float32),
        "out_w": rng.standard_normal((DIM, DIM), dtype=np.float32) * 0.02,
        "out_b": rng.standard_normal((DIM,), dtype=np.float32) * 0.02,
        "index": 1000,
    }
    out = kernel(**ins)
    print("out", out.shape, out.dtype, float(np.abs(out).mean()))


# revision 15
# speedup vs baseline: 1.0607x; 1.0607x over previous
"""Trainium2 Bass kernel for nn_MixedRepeatHeads.

Computation (full shapes):
  proj[h,b,k] = einsum(x[b,d], proj_w[h,k,d]) + proj_b[h,k]
  w = mix_w[:, index]; bb = mix_b[:, index]
  decay = clip(decay_value, 0.9, 1.0) ** (1/8)
  coef[h] = w*decay (h<8) else decay
  hidden[b, h*256+k] = w[h]*proj[h,b,k] + coef[h]*cache[h,b,k] + bb[h]
  out = hidden @ out_w.T + out_b                     # [8192, 4096]

Algebraic refold (host side, cheap):
  out = x @ C.T + (coef*cache) @ out_w.T + ob2
    C   = out_w @ (w*proj_w)          (one 4096^3 host GEMM)
    ob2 = out_b + out_w @ (w*proj_b + bb)
  => ONE GEMM with K=8192:  out = A @ Wcat + ob2
    A    = [x | coef*cache]           # [8192, 8192]
    Wcat = [C.T ; out_w.T]            # [8192, 4096]

Device: data-parallel over batch across 8 cores (1024 rows/core), all-bf16
matmuls (N=512 moving, fp32 PSUM accumulation; bf16 hides PE weight loads,
unlike f32r whose matmuls must self-load serially).  The batch is processed
as two 512-column halves, each a full pass over the 32 j-tiles streaming the
weights; each half's activation tile reload (WAR-ordered by the tile pool)
overlaps the opposite half's entire compute pass, so steady-state has no
activation stalls and DMA queues never head-of-line block the weight stream.
All DRAM layouts are partition-major so every DMA descriptor is a large
contiguous run per partition.
Device output is outT [4096, 1024] f32 per core; host transposes + concats.
"""

import sys

if "/opt/trn_rl_repo" not in sys.path:
    sys.path.insert(0, "/opt/trn_rl_repo")

import contextlib

import numpy as np
import ml_dtypes

import bass_rust
import concourse.bass as bass
import concourse.tile as tile
from concourse import mybir
from concourse.bass_utils import run_bass_kernel_spmd
from concourse.vector_clock import ScopedClock

# ---------------------------------------------------------------- constants
N_HEADS = 16
HIDDEN = 256
DIM = 4096
BATCH = 8192
DECAY_CONSTANT = 8
N_CORES = 8
BC = BATCH // N_CORES  # 1024 batch rows per core
HALF = BC // 2  # 512
P = 128
KDIM = 2 * DIM  # 8192 contraction (x ++ cache)
KT = KDIM // P  # 64 k-tiles
JT = DIM // P  # 32 j-tiles

F32 = mybir.dt.float32
BF16 = mybir.dt.bfloat16
NP_BF16 = ml_dtypes.bfloat16

# ------------------------------------------------- walrus wait legalization
# This walrus build supports only ONE sync-wait command per instruction.
MAXW = 1


class SafeTileContext(tile.TileContext):
    def _split_waits_in_ordered(self, ordered):
        nc = self.nc
        for _bb_name, insts in ordered.items():
            new_list = []
            changed = False
            for inst in insts:
                si = inst.sync_info
                if si is not None and len(si.on_wait) > MAXW:
                    waits = list(si.on_wait)
                    ups = list(si.on_update)
                    head, tail = waits[:-MAXW], waits[-MAXW:]
                    for w in head:
                        nop = mybir.InstNoOp(
                            name=nc.get_next_instruction_name(),
                            engine=inst.engine,
                            ins=[],
                            outs=[],
                            sync_info=bass_rust.SyncInfo(on_wait=[w], on_update=[]),
                            bass_nofuse=True,
                        )
                        nc.register_instruction(nop, overwrite=True)
                        new_list.append(nop)
                    inst.sync_info = bass_rust.SyncInfo(on_wait=tail, on_update=ups)
                    changed = True
                new_list.append(inst)
            if changed:
                insts[:] = new_list
        return ordered

    def _lower_ordered_insts(self, ordered):
        self._split_waits_in_ordered(ordered)
        return super()._lower_ordered_insts(ordered)

    def _drain_and_barrier(self, tick_clock, wait_clock):
        probe = self.nc.sync.nop(nofuse=True)
        wait_clock.add_sem_waits(
            probe.ins, ScopedClock({None: tick_clock.global_clock})
        )
        si = probe.ins.sync_info
        waits = list(si.on_wait) if si is not None else []
        upd = list(si.on_update) if si is not None else []
        probe.ins.sync_info = bass_rust.SyncInfo(on_wait=waits[:MAXW], on_update=upd)
        for i in range(MAXW, len(waits), MAXW):
            n = self.nc.sync.nop(nofuse=True)
            n.ins.sync_info = bass_rust.SyncInfo(
                on_wait=waits[i : i + MAXW], on_update=[]
            )

        self.nc.sync.drain()

        self.nc.all_engine_barrier()
        assert self.sems is not None
        popped = self.nc._tile_sem_poison_stack.pop()
        assert popped is self._sem_poison
        self.nc.clear_and_free_semaphores(list(self.sems.allocated().values()))
        self.nc.all_engine_barrier()


# ------------------------------------------------------------ kernel build
def build_kernel(loop_reps=None):
    """Per-core program. DRAM params:
      a    [2, P, KT, HALF] bf16 : a[h, p, kt, b] = A[h*512+b, kt*128+p]
      wt   [JT, P, KT, P]   bf16 : wt[jt, p, kt, j] = Wcat[kt*128+p, jt*128+j]
      ob   [P, JT]          f32  : ob[p, jt] = ob2[jt*128+p]
      outT [DIM, BC]        f32  : output, transposed (j-major)
    """
    nc = bass.Bass()
    a = nc.declare_dram_parameter("a", [2, P, KT, HALF], BF16, isOutput=False)
    wt = nc.declare_dram_parameter("wt", [JT, P, KT, P], BF16, isOutput=False)
    ob = nc.declare_dram_parameter("ob", [P, JT], F32, isOutput=False)
    outT = nc.declare_dram_parameter("outT", [DIM, BC], F32, isOutput=True)

    ACH = 8  # a-half DMA chunks (8 kt per chunk, contiguous per partition)
    KCH = KT // ACH

    with SafeTileContext(nc) as tc:
        with (
            tc.tile_pool(name="a0pool", bufs=1) as a0pool,
            tc.tile_pool(name="a1pool", bufs=1) as a1pool,
            tc.tile_pool(name="wpool", bufs=3) as wpool,
            tc.tile_pool(name="opool", bufs=3) as opool,
            tc.tile_pool(name="obpool", bufs=1) as obpool,
            tc.tile_pool(name="pspool", bufs=6, space="PSUM") as pspool,
        ):
            ob_t = obpool.tile([P, JT], F32)
            nc.sync.dma_start(ob_t[:], ob[:])

            loop_cm = (
                tc.For_i(0, loop_reps, 1)
                if loop_reps is not None
                else contextlib.nullcontext()
            )
            with loop_cm:
                for half in range(2):
                    apool = a0pool if half == 0 else a1pool
                    a_t = apool.tile([P, KT, HALF], BF16, tag=f"a{half}")
                    for c in range(ACH):
                        ksl = slice(c * KCH, (c + 1) * KCH)
                        nc.sync.dma_start(a_t[:, ksl, :], a[half][:, ksl, :])
                    for jt in range(JT):
                        wblk = wpool.tile([P, KT, P], BF16, tag="wblk")
                        nc.sync.dma_start(wblk[:], wt[jt])
                        ps = pspool.tile([P, HALF], F32, tag="ps")
                        for kt in range(KT):
                            nc.tensor.matmul(
                                ps[:],
                                wblk[:, kt, :],
                                a_t[:, kt, :],
                                start=(kt == 0),
                                stop=(kt == KT - 1),
                            )
                        o_t = opool.tile([P, HALF], F32, tag="o")
                        nc.vector.tensor_tensor(
                            o_t[:],
                            ps[:],
                            ob_t[:, jt : jt + 1].to_broadcast((P, HALF)),
                            mybir.AluOpType.add,
                        )
                        nc.sync.dma_start(
                            outT[
                                jt * P : (jt + 1) * P,
                                half * HALF : (half + 1) * HALF,
                            ],
                            o_t[:],
                        )

    return nc


# ------------------------------------------------------------- host helpers
def _host_prepare(inputs):
    x = np.asarray(inputs["x"], dtype=np.float32)
    proj_w = np.asarray(inputs["proj_w"], dtype=np.float32)
    proj_b = np.asarray(inputs["proj_b"], dtype=np.float32)
    mix_w = np.asarray(inputs["mix_w"], dtype=np.float32)
    mix_b = np.asarray(inputs["mix_b"], dtype=np.float32)
    decay_value = np.asarray(inputs["decay_value"], dtype=np.float32)
    cache = np.asarray(inputs["cache"], dtype=np.float32)
    out_w = np.asarray(inputs["out_w"], dtype=np.float32)
    out_b = np.asarray(inputs["out_b"], dtype=np.float32)
    idx = int(np.asarray(inputs["index"]))

    w = mix_w[:, idx]  # [16]
    bb = mix_b[:, idx]  # [16]
    decay = np.clip(decay_value, 0.9, 1.0) ** np.float32(1.0 / DECAY_CONSTANT)
    is_col = np.arange(N_HEADS) < (N_HEADS // 2)
    coef = np.where(is_col, w * decay, decay).astype(np.float32)  # [16]

    PW = (proj_w * w[:, None, None]).reshape(DIM, DIM)  # [i, d]
    C = out_w @ PW  # [j, d]
    bias_i = (w[:, None] * proj_b + bb[:, None]).reshape(DIM)  # [i]
    ob2 = out_b + out_w @ bias_i  # [j]

    # Wcat [K=8192, j]: rows 0..4095 = C.T, rows 4096.. = out_w.T
    Wcat = np.empty((KDIM, DIM), dtype=NP_BF16)
    Wcat[:DIM] = C.T.astype(NP_BF16)
    Wcat[DIM:] = out_w.T.astype(NP_BF16)
    WT = np.ascontiguousarray(Wcat.reshape(KT, P, JT, P).transpose(2, 1, 0, 3))

    # A.T [K=8192, batch]: rows 0..4095 = x.T, rows 4096.. = (coef*cache).T
    AT = np.empty((KDIM, BATCH), dtype=NP_BF16)
    AT[:DIM] = x.T.astype(NP_BF16)
    cc = (cache * coef[:, None, None]).astype(NP_BF16)  # [h, b, k]
    AT[DIM:] = cc.transpose(0, 2, 1).reshape(DIM, BATCH)
    # [kt, p, core, half, b'] -> per-core [2, P, KT, HALF] (partition-major)
    AT_r = AT.reshape(KT, P, N_CORES, 2, HALF).transpose(2, 3, 1, 0, 4)

    obT = np.ascontiguousarray(ob2.reshape(JT, P).T)  # [P, JT]

    in_maps = []
    for c in range(N_CORES):
        in_maps.append(
            {
                "a": np.ascontiguousarray(AT_r[c]),
                "wt": WT,
                "ob": obT,
            }
        )
    return in_maps


def _assemble(results):
    # results: list per core of {"outT": [DIM, BC]}
    out = np.empty((BATCH, DIM), dtype=np.float32)
    for c in range(N_CORES):
        out[c * BC : (c + 1) * BC] = results[c]["outT"].T
    return out


_NC_CACHE = None


def _get_nc():
    global _NC_CACHE
    if _NC_CACHE is None:
        _NC_CACHE = build_kernel()
    return _NC_CACHE


def kernel(**inputs) -> np.ndarray:
    in_maps = _host_prepare(inputs)
    nc = _get_nc()
    res = run_bass_kernel_spmd(nc, in_maps, list(range(N_CORES)))
    return _assemble(res.results)


if __name__ == "__main__":
    # quick self-run with random data of the right shapes
    rng = np.random.default_rng(0)
    ins = {
        "x": rng.standard_normal((BATCH, DIM), dtype=np.float32),
        "proj_w": rng.standard_normal((N_HEADS, HIDDEN, DIM), dtype=np.float32) * 0.02,
        "proj_b": rng.standard_normal((N_HEADS, HIDDEN), dtype=np.float32) * 0.02,
        "mix_w": rng.standard_normal((N_HEADS, 4096), dtype=np.float32) * 0.02 + 1.0,
        "mix_b": rng.standard_normal((N_HEADS, 4096), dtype=np.float32) * 0.02,
        "decay_value": rng.uniform(0.85, 1.05, size=(N_HEADS,)).astype(np.float32),
        "cache": rng.standard_normal((N_HEADS, BATCH, HIDDEN), dtype=np.float32),
        "out_w": rng.standard_normal((DIM, DIM), dtype=np.float32) * 0.02,
        "out_b": rng.standard_normal((DIM,), dtype=np.float32) * 0.02,
        "index": 1000,
    }
    out = kernel(**ins)
    print("out", out.shape, out.dtype, float(np.abs(out).mean()))
